# revision 37
# baseline (speedup 1.0000x reference)
"""BiMPM matching kernel for Trainium2 (Bass/Tile), 8 NeuronCores.

Strategy: data-parallel over batch (B=8 -> one batch per core). Per core:
  c1, c2: (256, 128) fp32 (forward half of the contexts). All masks are
  all-ones in this problem (verified on host; numpy fallback otherwise).

v5 — restructured around three ideas:

1. ext-21 weights. Every per-perspective weight matrix gets a leading
   all-ones column ([1 | w^2] -> (h, 21)), so the unweighted "single"
   cosine and the 20-perspective "multi" match come out of ONE matmul,
   land in adjacent output columns, and share one reciprocal-norm tail.
   The same column-0 trick folds the plain cosine norms (r1c/r2c) and the
   cosine mean into the maxpool norm block / mean matmuls for free.

2. Batched rsqrt. All weighted-norm matmuls for one weight set (4x
   [128,21], both sides x both chunks) write one PSUM bank; a single
   Ln + Exp(-0.5 x) pair rsqrts the whole [128,84] bank. The ~185ns
   fixed SBUF-access cost of ACT ops is paid 14x instead of 80x.

3. lambda-ladder scans. The maxpool fused multiply+max DVE scan is the
   irreducible backbone (2.6M products/core through a 1 elem/cycle
   engine). Four perspectives {g, g+5, g+10, g+15} are packed into ONE
   [128, 4*256] scan: broadcast row k is pre-scaled by LAM^k (LAM=2^30,
   exact in the fp32 exponent), so each segment's running max dominates
   everything before it and the per-segment max is recovered exactly at
   the segment's last column (descaled by LAM^-k at harvest). 80 scans
   become 20, and the per-op PSUM-access + dispatch overhead amortizes
   4x. Scan outputs are bf16 (range 2^90 overflows f16).

Also: one perspective-scale per p (w (.) c2T serves both matmul sides),
packed input DMAs (3 loads), single [S,210] output tensor, power-mean
max-attentive path as in v4 (K=16, sign-split relu chains), engines
balanced DVE ~ scans+chains / ACT ~ transcendentals+relus+u-powers /
Pool ~ scales+X-mults+reduces.

Feature columns per side: 0 cosmax | 1 cosmean | 2 full_single |
3:23 full_multi | 23:43 mp_max | 43:63 mp_mean | 63 att_single |
64:84 att_multi | 84 amax_single | 85:105 amax_multi
"""

import numpy as np

EPS = 1e-7
S = 256  # sequence length (s1 == s2)
H = 128  # forward hidden size
P = 20   # perspectives
PX = 21  # ext: [ones | w^2]
B = 8    # batch == n_cores
C_MEAN = np.float32(1.0 / (256.0 + EPS))  # masked_mean divisor (all-ones masks)

ALPHA = 8.0   # u-side scale in the power-mean (underflow guard)
VG = 4.5      # v-side normalizer (bound on |randn| at this sample count)
KPOW = 16     # power-mean order; quotient uses S_16 and S_32
LAM = 2.0 ** 30  # maxpool scan segment ladder (exact power of two)

_CACHE = {}

_SCAN_NAME = "ANT_TTMAX_SCAN_V3"


def _register_scan():
    """Custom DVE op: out[p,k] = running max of in0[p,:k+1]*in1[p,:k+1].

    Regular mode only (the 2X table slots measurably drop odd elements on
    TRN2 silicon). out[:, -1] is the full fused multiply+max reduction; no
    accumulator companion instruction is emitted. The stock
    tensor_tensor_reduce ISA opcode has no TRN2 table row (device crash);
    the ant custom-DVE table is the supported path.
    """
    from concourse.dve_ops import DveOp, OPS, CUSTOM_DVE_SPECS, \
        _SUB_OPCODE_FOR_NAME, _CUSTOM_DVE_ROW_BASE
    from concourse.dve_spec import Spec, Src0, Src1, scan, lower, _has_src1, \
        AluOp
    from concourse.dve_uop import DveOpSpec

    if _SCAN_NAME in _SUB_OPCODE_FOR_NAME:
        return next(op for op in OPS if op.name == _SCAN_NAME)

    def _ref(in0, in1, c0, c1, c2):
        b = (np.asarray(in0, np.float32) * np.asarray(in1, np.float32))
        b = b.astype(np.float32)
        P_ = b.shape[0]
        return np.maximum.accumulate(b.reshape(P_, -1), axis=1)

    spec = Spec(body=scan(AluOp.MAX, Src0 * Src1), reference=_ref)
    row = _CUSTOM_DVE_ROW_BASE + len(OPS)
    assert row < 0x20
    shas = {}
    for ver in ("v3", "v4"):
        tmp = DveOpSpec(name=_SCAN_NAME, opcode=row,
                        uops=lower(spec, ver=ver), rd1_en=_has_src1(spec))
        shas[ver] = tmp.sha(ver)
    op = DveOp(_SCAN_NAME, spec, subdim=False, uops_sha=shas)
    OPS.append(op)
    _SUB_OPCODE_FOR_NAME[op.name] = row
    CUSTOM_DVE_SPECS[op.name] = spec
    return op


def _build_program(n_cores=8):
    import concourse.bacc as bacc
    import concourse.tile as tile
    import concourse.mybir as mybir
    import concourse.bass as bass_mod
    from concourse.masks import make_identity
    import concourse.hw_specs as hw_specs

    # Every ACT function this kernel uses (Exp, Ln, Copy, Square, Relu,
    # Identity) lives together in the "natural_log_exp_and_others" set; the
    # default per-function set chooser picks the first containing set and
    # thrashes a 1.3us table reload on every transition. Restrict the
    # choices to the combined set for this build.
    _orig_gat = hw_specs.get_activation_tables

    def _gat_combined(module_arch):
        tabs = _orig_gat(module_arch)
        keep = "natural_log_exp_and_others"
        assert keep in tabs
        return {k: (v if k == keep else set()) for k, v in tabs.items()}

    hw_specs.get_activation_tables = _gat_combined
    bacc.get_activation_tables = _gat_combined
    try:
        return _build_program_inner(n_cores, bacc, tile, mybir, bass_mod,
                                    make_identity)
    finally:
        hw_specs.get_activation_tables = _orig_gat
        bacc.get_activation_tables = _orig_gat


def _build_program_inner(n_cores, bacc, tile, mybir, bass_mod, make_identity):
    import math

    f32 = mybir.dt.float32
    f32r = mybir.dt.float32r
    bf16 = mybir.dt.bfloat16
    AL = mybir.AluOpType
    AF = mybir.ActivationFunctionType
    AX = mybir.AxisListType
    AP = bass_mod.AP

    scan_op = _register_scan()

    nc = bacc.Bacc("TRN2", target_bir_lowering=False, debug=False,
                   num_devices=n_cores)

    # ---- DRAM I/O (per core) ----
    ctp_d = nc.dram_tensor("ctp", [H, 2 * S], f32, kind="ExternalInput").ap()
    cp_d = nc.dram_tensor("cp", [128, 4, H], f32, kind="ExternalInput").ap()
    wxf_d = nc.dram_tensor("wxf", [H, 4, PX], f32, kind="ExternalInput").ap()
    out_d = nc.dram_tensor("out", [S, 210], f32, kind="ExternalOutput").ap()

    with tile.TileContext(nc) as tc:
        with tc.tile_pool(name="sb", bufs=1) as sb, \
             tc.tile_pool(name="sbr", bufs=4) as sbr, \
             tc.tile_pool(name="ps_num", bufs=2, space="PSUM") as _ps_num, \
             tc.tile_pool(name="ps_nb", bufs=2, space="PSUM") as _ps_nb, \
             tc.tile_pool(name="ps_gen", bufs=2, space="PSUM") as _ps_gen, \
             tc.tile_pool(name="dram_scratch", bufs=1, space="DRAM") as dsc:

            # PSUM tiles pad to full banks; one tag per pool so the bank
            # budget stays fixed: num 2x[128,1024] = 4 banks, nb 2x1, gen 2x1.
            class _TaggedPool:
                def __init__(self, pool, tag):
                    self.pool, self.tag = pool, tag

                def tile(self, shape, dtype):
                    return self.pool.tile(shape, dtype, tag=self.tag,
                                          name=self.tag)

            ps_num = _TaggedPool(_ps_num, "num")
            ps_nb = _TaggedPool(_ps_nb, "nb")
            ps_gen = _TaggedPool(_ps_gen, "gen")

            def scan_max(in0, in1, out):
                return nc.vector._custom_dve(scan_op, out=out, in0=in0,
                                             in1=in1)

            def flat(t, n):
                """[128, n] view of a tile's first n free elements."""
                a = t[:]
                return AP(tensor=a.tensor, offset=a.offset,
                          ap=[list(a.ap[0]), [1, n]])

            def stride_view(t, off, stride, count):
                a = t if isinstance(t, AP) else t[:]
                return AP(tensor=a.tensor, offset=a.offset + off,
                          ap=[list(a.ap[0]), [stride, count]])

            # ================= loads & constants =================
            ctp = sb.tile([H, 2 * S], f32)
            nc.sync.dma_start(ctp[:], ctp_d)
            cp = sb.tile([128, 4, H], f32)
            nc.sync.dma_start(cp[:], cp_d)
            wxf = sb.tile([H, 4, PX], f32)
            nc.sync.dma_start(wxf[:], wxf_d)
            c1T = ctp[:, 0:S]
            c2T = ctp[:, S:2 * S]

            ones_row = sb.tile([1, 128], f32)
            nc.vector.memset(ones_row[:], 1.0)
            ones_col = sb.tile([128, 1], f32)
            nc.vector.memset(ones_col[:], 1.0)
            ident = sb.tile([128, 128], f32)
            make_identity(nc, ident[:])
            QBIAS = float(math.log(VG / ALPHA))
            qbias_col = sb.tile([128, 1], f32)
            nc.vector.memset(qbias_col[:], QBIAS)
            zero_col = sb.tile([128, 1], f32)
            nc.vector.memset(zero_col[:], 0.0)
            lrowp = sb.tile([128, P], f32)
            lrow = sb.tile([128, P], f32)
            for k in range(4):
                nc.vector.memset(lrowp[:, 5 * k:5 * (k + 1)], LAM ** k)
                nc.vector.memset(lrow[:, 5 * k:5 * (k + 1)], LAM ** (-k))

            out_a = sb.tile([128, 2, 210], f32)
            nc.vector.memset(out_a[:], 0.0)
            out_all = [out_a[:, 0, :], out_a[:, 1, :]]

            # PE pstate warm-up: dependency-free chain long enough to bridge
            # the input-DMA wait so the first real (fp32) matmuls run at full
            # clock.
            for _ in range(7):
                wt = ps_gen.tile([1, 128], f32)
                nc.tensor.matmul(wt[:], ones_row[0:1, 0:1], ones_row[:],
                                 start=True, stop=True)

            # f32r-rounded copies for the cosine dot products (f32r
            # streams 1 cycle/row at free >= 256; 4x over plain fp32)
            c1Tr = sb.tile([H, S], f32r)
            c2Tr = sb.tile([H, S], f32r)
            nc.vector.tensor_copy(c1Tr[:], c1T)
            nc.vector.tensor_copy(c2Tr[:], c2T)
            pml = []
            for c in range(2):
                pm = ps_num.tile([128, 4 * S], f32)
                nc.tensor.matmul(pm[:, 0:S], c1Tr[:, c * 128:(c + 1) * 128],
                                 c2Tr[:], start=True, stop=True)
                nc.tensor.matmul(pm[:, S:2 * S], c2Tr[:, c * 128:(c + 1) * 128],
                                 c1Tr[:], start=True, stop=True)
                pml.append(pm)

            # bf16 copies (matmul operands)
            c1Tb = sb.tile([H, S], bf16)
            c2Tb = sb.tile([H, S], bf16)
            nc.gpsimd.tensor_copy(c1Tb[:], c1T)
            nc.gpsimd.tensor_copy(c2Tb[:], c2T)
            wxb = sb.tile([H, 4, PX], bf16)
            nc.gpsimd.tensor_copy(wxb[:], wxf[:])

            # squares (f32: norms feed every cosine denominator)
            c1sqT = sb.tile([H, S], f32)
            c2sqT = sb.tile([H, S], f32)
            nc.scalar.activation(c1sqT[:], c1T, AF.Square)
            nc.scalar.activation(c2sqT[:], c2T, AF.Square)

            sq_chunks = [c1sqT[:, 0:128], c1sqT[:, 128:256],
                         c2sqT[:, 0:128], c2sqT[:, 128:256]]

            # ================= batched norm blocks =================
            # R[w] = rsqrt of [c1sq_c0|c1sq_c1|c2sq_c0|c2sq_c1] x w_ext,
            # one PSUM bank + one Ln/Exp pair per weight set. Only the
            # maxpool block is on the critical path (rT -> lin -> bc ->
            # scans); the f/a/m blocks are woven into the backbone.
            def norm_block(widx, tag):
                pw = ps_nb.tile([128, 4 * PX], f32)
                for s_ in range(4):
                    nc.tensor.matmul(pw[:, s_ * PX:(s_ + 1) * PX],
                                     sq_chunks[s_], wxf[:, widx, :],
                                     start=True, stop=True)
                ln = sbr.tile([128, 4 * PX], f32, tag=f"ln{tag}", name="t",
                              bufs=2)
                nc.scalar.activation(ln[:], pw[:], AF.Ln)
                r = sb.tile([128, 4 * PX], f32, tag=f"R{tag}", name=f"R{tag}")
                nc.scalar.activation(r[:], ln[:], AF.Exp, scale=-0.5)
                return r

            R_mp = norm_block(1, "mp")   # also r1c/r2c in cols 0 of each 21

            def Rsl(r, s_, lo=0, hi=PX):
                return r[:, s_ * PX + lo:s_ * PX + hi]

            # ================= maxpool setup (critical chain) =================
            # rT[p, side*S + j] = rsqrt-weighted-norm, partition-block lambda
            # ladder baked in; one bank of transposes, one ladder-multiply,
            # one DRAM store for the broadcast loads.
            hp_mp = tc.high_priority()
            hp_mp.__enter__()
            rsc = sb.tile([128, 4 * P], f32)
            for s_ in range(4):
                nc.vector.tensor_tensor(out=rsc[:, s_ * P:(s_ + 1) * P],
                                        in0=Rsl(R_mp, s_, 1, PX),
                                        in1=lrowp[:], op=AL.mult)
            ptb = ps_gen.tile([P, 4 * 128], f32)
            for s_ in range(4):
                nc.tensor.transpose(ptb[:, s_ * 128:(s_ + 1) * 128],
                                    rsc[:, s_ * P:(s_ + 1) * P], ident[:])
            rTb = sb.tile([P, 2 * S], bf16)
            nc.scalar.copy(rTb[:], ptb[:])
            lin = dsc.tile([P, 2 * S], bf16, tag="lin", name="t")
            nc.sync.dma_start(lin[:], rTb[:])
            hp_mp.__exit__(None, None, None)

            # row norms for the cosine (1/|c1_i|, 1/|c2_j| as [1,S] rows)
            prow = ps_nb.tile([1, 2 * S], f32)
            nc.tensor.matmul(prow[0:1, 0:S], ones_col[:], c1sqT[:],
                             start=True, stop=True)
            nc.tensor.matmul(prow[0:1, S:2 * S], ones_col[:], c2sqT[:],
                             start=True, stop=True)
            lnrow = sbr.tile([1, 2 * S], f32, tag="lnrow", name="t", bufs=1)
            nc.scalar.activation(lnrow[:], prow[:], AF.Ln)
            rows_r = sb.tile([1, 2 * S], f32)
            nc.scalar.activation(rows_r[:], lnrow[:], AF.Exp, scale=-0.5)

            bc_p = ps_gen.tile([128, 2 * S], f32)
            nc.tensor.matmul(bc_p[:, 0:S], ones_row[:], rows_r[0:1, 0:S],
                             start=True, stop=True)
            nc.tensor.matmul(bc_p[:, S:2 * S], ones_row[:], rows_r[0:1, S:2 * S],
                             start=True, stop=True)
            bc_r = sb.tile([128, 2 * S], f32)
            nc.scalar.copy(bc_r[:], bc_p[:])

            cos = [sb.tile([128, S], f32, tag=f"cos{c}", name=f"cos{c}") for c in range(2)]
            cosT = [sb.tile([128, S], f32, tag=f"cosT{c}", name=f"cosT{c}") for c in range(2)]
            for c in range(2):
                nc.vector.scalar_tensor_tensor(
                    out=cos[c][:], in0=pml[c][:, 0:S],
                    scalar=Rsl(R_mp, c, 0, 1), in1=bc_r[:, S:2 * S],
                    op0=AL.mult, op1=AL.mult)
                nc.vector.scalar_tensor_tensor(
                    out=cosT[c][:], in0=pml[c][:, S:2 * S],
                    scalar=Rsl(R_mp, 2 + c, 0, 1), in1=bc_r[:, 0:S],
                    op0=AL.mult, op1=AL.mult)
                nc.vector.reduce_max(out=out_all[c][:, 0:1], in_=cos[c][:],
                                     axis=AX.X)
                nc.vector.reduce_max(out=out_all[c][:, 105:106], in_=cosT[c][:],
                                     axis=AX.X)

            # descale tiles: rD = R_slice * LAM^-(p//5), per side x chunk
            rD = []
            for s_ in range(4):
                t = sb.tile([128, P], f32, tag=f"rd{s_}", name="t")
                nc.vector.tensor_tensor(out=t[:], in0=Rsl(R_mp, s_, 1, PX),
                                        in1=lrow[:], op=AL.mult)
                rD.append(t)

            def bcast_dma(g, side):
                # side 0 loads the c2-side rT rows (lin cols S:2S)
                src = lin[:]
                t = sbr.tile([128, 4 * S], bf16, tag=f"bcd{side}", name="t",
                             bufs=2)
                nc.sync.dma_start(t[:], AP(
                    tensor=src.tensor,
                    offset=src.offset + g * 2 * S + (1 - side) * S,
                    ap=[[0, 128], [10 * S, 4], [1, S]]))
                return t

            def c2scale(p):
                t = sbr.tile([H, S], bf16, tag="c2s", name="t", bufs=20)
                nc.gpsimd.tensor_scalar_mul(t[:], c2T,
                                            wxf[:, 1, 1 + p:2 + p])
                return t

            def mp_nums(g, c2s_g):
                """16 matmuls for group g -> 4 combo tiles [128, 4*256]."""
                tiles = []
                for combo in range(4):
                    side, c = combo // 2, combo % 2
                    pn = ps_num.tile([128, 4 * S], f32)
                    for k in range(4):
                        cs = c2s_g[k]
                        if side == 0:
                            nc.tensor.matmul(pn[:, k * S:(k + 1) * S],
                                             c1Tb[:, c * 128:(c + 1) * 128],
                                             cs[:], start=True, stop=True)
                        else:
                            nc.tensor.matmul(pn[:, k * S:(k + 1) * S],
                                             cs[:, c * 128:(c + 1) * 128],
                                             c1Tb[:], start=True, stop=True)
                    tiles.append(pn)
                return tiles

            def mp_scan_harvest(g, pns, bc):
                for combo in range(4):
                    side, c = combo // 2, combo % 2
                    so = sbr.tile([128, 4 * S], bf16, tag=f"so{combo}",
                                  name="t", bufs=2)
                    scan_max(flat(pns[combo], 4 * S), flat(bc[side], 4 * S),
                             so[:])
                    base = 105 * side + 23 + g
                    nc.vector.tensor_tensor(
                        out=stride_view(out_all[c], base, 5, 4),
                        in0=stride_view(so, S - 1, S, 4),
                        in1=stride_view(rD[combo], g, 5, 4), op=AL.mult)

            # ================= glue generators =================
            R = {"mp": R_mp}

            def g_norm_rest():
                R["f"] = norm_block(0, "f")
                yield
                R["a"] = norm_block(2, "a")
                yield
                R["m"] = norm_block(3, "m")
                yield

            def g_chain_v():
                """v-chains: relu(+-c/VG) on DVE, ^2..^32 on ACT."""
                for side, src_off in ((0, 2), (1, 0)):  # v1 from c2, v2 from c1
                    a = sb.tile([128, 4 * H], bf16, tag=f"v{side}a", name="t")
                    b = sb.tile([128, 4 * H], bf16, tag=f"v{side}b", name="t")
                    for q in range(4):
                        c, s_ = q // 2, q % 2
                        sc = (1.0 / VG) * (1 if s_ == 0 else -1)
                        nc.vector.tensor_scalar(
                            out=a[:, q * H:(q + 1) * H],
                            in0=cp[:, src_off + c, :], scalar1=sc,
                            scalar2=0.0, op0=AL.mult, op1=AL.max)
                        yield
                    cur, nxt = a, b
                    for _ in range(5):
                        nc.scalar.activation(nxt[:], cur[:], AF.Square)
                        cur, nxt = nxt, cur
                        yield
                    # 5 squarings, ping-pong: a holds x^16, b holds x^32
                    _chains[f"v{side}16"] = a
                    _chains[f"v{side}32"] = b

            def g_chain_u(side):
                """u-chains: relu(+-ALPHA*cos^T) on DVE, powers on ACT."""
                srcs = cosT if side == 0 else cos
                a = sb.tile([128, 4 * S], bf16, tag=f"u{side}a", name="t")
                b = sb.tile([128, 4 * S], bf16, tag=f"u{side}b", name="t")
                for q in range(4):
                    c, s_ = q // 2, q % 2
                    sc = ALPHA * (1 if s_ == 0 else -1)
                    nc.vector.tensor_scalar(
                        out=a[:, q * S:(q + 1) * S], in0=srcs[c][:],
                        scalar1=sc, scalar2=0.0, op0=AL.mult, op1=AL.max)
                    yield
                cur, nxt = a, b
                for _ in range(5):
                    nc.scalar.activation(nxt[:], cur[:], AF.Square)
                    cur, nxt = nxt, cur
                    yield
                _chains[f"u{side}16"] = a
                _chains[f"u{side}32"] = b

            _chains = {}

            def g_power(side, out_t):
                """(S32/S16)^(1/16)*VG/ALPHA in transposed (h, i) layout."""
                u16, u32 = _chains[f"u{side}16"], _chains[f"u{side}32"]
                v16, v32 = _chains[f"v{side}16"], _chains[f"v{side}32"]
                lns = []
                for lvl, (uk, vk) in enumerate(((u16, v16), (u32, v32))):
                    sp = ps_nb.tile([128, S], f32)
                    for q in range(4):
                        nc.tensor.matmul(sp[:], vk[:, q * H:(q + 1) * H],
                                         uk[:, q * S:(q + 1) * S],
                                         start=(q == 0), stop=(q == 3))
                    ln = sbr.tile([128, S], f32, tag=f"pln{lvl}", name="t",
                                  bufs=2)
                    nc.scalar.activation(ln[:], sp[:], AF.Ln)
                    lns.append(ln)
                    yield
                diff = sbr.tile([128, S], f32, tag="pdif", name="t", bufs=2)
                nc.gpsimd.tensor_sub(diff[:], lns[1][:], lns[0][:])
                nc.scalar.activation(out_t[:], diff[:], AF.Exp,
                                     scale=1.0 / KPOW, bias=qbias_col[:])
                yield

            def g_softmax(side):
                """att softmax; side 0: att2 (v1 side), 1: att1."""
                cosrows = cosT if side == 0 else cos
                coff = 2 if side == 0 else 0
                tgt = att2T if side == 0 else att1T
                for c in range(2):
                    pp = ps_gen.tile([128, H], f32)
                    for jc in range(2):
                        nc.tensor.matmul(pp[:],
                                         cosrows[jc][:, c * 128:(c + 1) * 128],
                                         cp[:, coff + jc, :],
                                         start=(jc == 0), stop=(jc == 1))
                    se = sbr.tile([128, 1], f32, tag="sse", name="t")
                    ex = sbr.tile([128, H], f32, tag="sex", name="t", bufs=2)
                    nc.scalar.activation(ex[:], pp[:], AF.Exp,
                                         bias=zero_col[:], scale=1.0,
                                         accum_out=se[:])
                    yield
                    rse = sbr.tile([128, 1], f32, tag="srse", name="t")
                    nc.vector.reciprocal_approx_fast(rse[:], se[:])
                    ex2 = sbr.tile([128, H], f32, tag="sex2", name="t",
                                   bufs=2)
                    nc.scalar.mul(ex2[:], ex[:], rse[:])
                    pt = ps_gen.tile([128, 128], f32)
                    nc.tensor.transpose(pt[:], ex2[:], ident[:])
                    nc.scalar.copy(tgt[:, c * 128:(c + 1) * 128], pt[:])
                    yield

            att2T = sb.tile([H, S], f32)
            att1T = sb.tile([H, S], f32)
            amax2T = sb.tile([H, S], f32)
            amax1T = sb.tile([H, S], f32)

            def g_pos_match(side, t2T, widx, rkey, col0):
                """ext pos-match: X=(c self).(t2), singles fused via col 0."""
                cT_self = c1T if side == 0 else c2T
                xb = sbr.tile([H, S], bf16, tag="xb", name="t", bufs=2)
                nc.gpsimd.tensor_mul(xb[:], cT_self, t2T[:])
                t2sq = sbr.tile([H, S], bf16, tag="t2sq", name="t", bufs=2)
                nc.gpsimd.tensor_mul(t2sq[:], t2T[:], t2T[:])
                yield
                pn2 = ps_nb.tile([128, 2 * PX], f32)
                for c in range(2):
                    nc.tensor.matmul(pn2[:, c * PX:(c + 1) * PX],
                                     t2sq[:, c * 128:(c + 1) * 128],
                                     wxb[:, widx, :], start=True, stop=True)
                ln2 = sbr.tile([128, 2 * PX], f32, tag="pmln", name="t",
                               bufs=2)
                nc.scalar.activation(ln2[:], pn2[:], AF.Ln)
                rsqt2 = sbr.tile([128, 2 * PX], f32, tag="pmrs", name="t",
                                 bufs=2)
                nc.scalar.activation(rsqt2[:], ln2[:], AF.Exp, scale=-0.5)
                yield
                for c in range(2):
                    pnum = ps_gen.tile([128, PX], f32)
                    nc.tensor.matmul(pnum[:], xb[:, c * 128:(c + 1) * 128],
                                     wxb[:, widx, :], start=True, stop=True)
                    den = sbr.tile([128, PX], f32, tag="pmden", name="t",
                                   bufs=2)
                    nc.gpsimd.tensor_mul(den[:],
                                         rsqt2[:, c * PX:(c + 1) * PX],
                                         Rsl(R[rkey], 2 * side + c))
                    nc.vector.tensor_tensor(
                        out=out_all[c][:, 105 * side + col0:
                                       105 * side + col0 + PX],
                        in0=pnum[:], in1=den[:], op=AL.mult)
                    yield

            def g_full():
                # last-row ([1,21]) weighted norms of both sides, one bank +
                # one rsq pair (matmul operands must sit at base partition 0)
                plast = ps_nb.tile([1, 2 * PX], f32)
                nc.tensor.matmul(plast[0:1, 0:PX], c2sqT[:, S - 1:S],
                                 wxf[:, 0, :], start=True, stop=True)
                nc.tensor.matmul(plast[0:1, PX:2 * PX], c1sqT[:, S - 1:S],
                                 wxf[:, 0, :], start=True, stop=True)
                lnl = sbr.tile([1, 2 * PX], f32, tag="flnl", name="t")
                nc.scalar.activation(lnl[:], plast[:], AF.Ln)
                lastr = sbr.tile([1, 2 * PX], f32, tag="flast", name="t")
                nc.scalar.activation(lastr[:], lnl[:], AF.Exp, scale=-0.5)
                yield
                for side in range(2):
                    cTb_s = c1Tb if side == 0 else c2Tb
                    cT_o = c2T if side == 0 else c1T
                    rhs = sbr.tile([H, PX], bf16, tag="frhs", name="t", bufs=2)
                    nc.vector.tensor_scalar(out=rhs[:], in0=wxb[:, 0, :],
                                            scalar1=cT_o[:, S - 1:S],
                                            scalar2=None, op0=AL.mult)
                    bcf = ps_gen.tile([128, PX], f32)
                    nc.tensor.matmul(bcf[:], ones_row[:],
                                     lastr[0:1, side * PX:(side + 1) * PX],
                                     start=True, stop=True)
                    bcs = sbr.tile([128, PX], f32, tag="fbcs", name="t",
                                   bufs=2)
                    nc.vector.tensor_copy(bcs[:], bcf[:])
                    yield
                    for c in range(2):
                        pnum = ps_gen.tile([128, PX], f32)
                        nc.tensor.matmul(pnum[:],
                                         cTb_s[:, c * 128:(c + 1) * 128],
                                         rhs[:], start=True, stop=True)
                        den = sbr.tile([128, PX], f32, tag="fden", name="t",
                                       bufs=2)
                        nc.gpsimd.tensor_mul(den[:], Rsl(R["f"], 2 * side + c),
                                             bcs[:])
                        nc.vector.tensor_tensor(
                            out=out_all[c][:, 105 * side + 2:105 * side + 23],
                            in0=pnum[:], in1=den[:], op=AL.mult)
                        yield

            def g_mp_mean():
                for side in range(2):
                    # g2T_ext[h, q] = sum_j cother[j,h] * R_other[j, q]
                    coff = 2 if side == 0 else 0
                    cTb_s = c1Tb if side == 0 else c2Tb
                    gT = ps_nb.tile([128, PX], f32)
                    for c in range(2):
                        nc.tensor.matmul(gT[:], cp[:, coff + c, :],
                                         Rsl(R_mp, (1 - side) * 2 + c),
                                         start=(c == 0), stop=(c == 1))
                    wg = sbr.tile([H, PX], bf16, tag="wg", name="t", bufs=2)
                    nc.vector.tensor_tensor(out=wg[:], in0=wxb[:, 1, :],
                                            in1=gT[:], op=AL.mult)
                    yield
                    for c in range(2):
                        pmx = ps_gen.tile([128, PX], f32)
                        nc.tensor.matmul(pmx[:],
                                         cTb_s[:, c * 128:(c + 1) * 128],
                                         wg[:], start=True, stop=True)
                        nc.vector.scalar_tensor_tensor(
                            out=out_all[c][:, 105 * side + 43:
                                           105 * side + 63],
                            in0=pmx[:, 1:PX], scalar=float(C_MEAN),
                            in1=Rsl(R_mp, 2 * side + c, 1, PX),
                            op0=AL.mult, op1=AL.mult)
                        nc.vector.scalar_tensor_tensor(
                            out=out_all[c][:, 105 * side + 1:
                                           105 * side + 2],
                            in0=pmx[:, 0:1], scalar=float(C_MEAN),
                            in1=Rsl(R_mp, 2 * side + c, 0, 1),
                            op0=AL.mult, op1=AL.mult)
                        yield

            def out_dst(lo, n):
                a = out_d
                return AP(tensor=a.tensor, offset=a.offset + lo,
                          ap=[[210, 128], [128 * 210, 2], [1, n]])

            def g_store_early(side):
                lo = 105 * side + 43
                nc.sync.dma_start(out_dst(lo, 62), out_a[:, :, lo:lo + 62])
                yield

            # ================= weave =================
            from collections import deque
            pending = deque()

            def pump(n):
                for _ in range(n):
                    emitted = False
                    while pending and not emitted:
                        g_ = pending.popleft()
                        try:
                            next(g_)
                            pending.insert(min(2, len(pending)), g_)
                            emitted = True
                        except StopIteration:
                            pass
                    if not pending and not emitted:
                        return

            def run_all(gen):
                for _ in gen:
                    pass

            # schedule: per-group event lists (run between scan groups)
            events = {
                0: [g_norm_rest(), g_chain_v(), g_chain_u(0), g_softmax(0)],
                1: [g_chain_u(1), g_full(), g_pos_match(0, att2T, 2, "a", 63)],
                2: [g_power(0, amax2T), g_softmax(1), g_mp_mean()],
                3: [g_pos_match(0, amax2T, 3, "m", 84), g_power(1, amax1T),
                    g_pos_match(1, att1T, 2, "a", 63)],
                4: [g_pos_match(1, amax1T, 3, "m", 84)],
            }

            # ---- maxpool backbone ----
            bc = (bcast_dma(0, 0), bcast_dma(0, 1))
            c2s_g = [c2scale(0 + 5 * k) for k in range(4)]
            pns = mp_nums(0, c2s_g)
            for g in range(5):
                for ev in events.get(g, ()):
                    if callable(ev):
                        ev()
                    else:
                        pending.append(ev)
                if g + 1 < 5:
                    nbc = (bcast_dma(g + 1, 0), bcast_dma(g + 1, 1))
                    c2s_n = [c2scale(g + 1 + 5 * k) for k in range(4)]
                pump(8)
                mp_scan_harvest(g, pns, bc)
                if g + 1 < 5:
                    pns = mp_nums(g + 1, c2s_n)
                    bc = nbc
                pump(10)
                if g == 4:
                    # drain all glue, then the early stores (their columns
                    # must be fully emitted before the DMA reads them)
                    pump(100000)
                    run_all(g_store_early(0))
                    run_all(g_store_early(1))
            pump(1000)

            # ================= final stores =================
            nc.sync.dma_start(out_dst(0, 43), out_a[:, :, 0:43])
            nc.sync.dma_start(out_dst(105, 43), out_a[:, :, 105:148])

    nc.finalize()
    return nc


def _get_program(n_cores=8):
    key = ("prog", n_cores)
    if key not in _CACHE:
        _CACHE[key] = _build_program(n_cores)
    return _CACHE[key]


def _get_runner(n_cores=8):
    """Build (once) a cached jitted executor: fn(in_maps) -> per-core outputs.

    Mirrors concourse.bass2jax.run_bass_via_pjrt's multi-core path, but keeps
    the jitted shard_map so repeat calls skip tracing/compile-cache lookups.
    """
    key = ("runner", n_cores)
    if key in _CACHE:
        return _CACHE[key]

    import jax
    import numpy as _np
    from jax.experimental.shard_map import shard_map
    from jax.sharding import Mesh, PartitionSpec
    import concourse.mybir as mybir
    from concourse.bass2jax import (_bass_exec_p, install_neuronx_cc_hook,
                                    partition_id_tensor)

    nc = _get_program(n_cores)
    install_neuronx_cc_hook()
    partition_name = (nc.partition_id_tensor.name
                      if nc.partition_id_tensor else None)

    in_names, out_names, out_shapes, out_dtypes = [], [], [], []
    for alloc in nc.m.functions[0].allocations:
        if not isinstance(alloc, mybir.MemoryLocationSet):
            continue
        name = alloc.memorylocations[0].name
        if alloc.kind == "ExternalInput":
            if name != partition_name:
                in_names.append(name)
        elif alloc.kind == "ExternalOutput":
            out_names.append(name)
            out_shapes.append(tuple(alloc.tensor_shape))
            out_dtypes.append(mybir.dt.np(alloc.dtype))
    n_params = len(in_names)
    n_outs = len(out_names)
    out_avals = [jax.core.ShapedArray(s, d)
                 for s, d in zip(out_shapes, out_dtypes)]
    all_in_names = list(in_names) + list(out_names)
    if partition_name is not None:
        all_in_names.append(partition_name)

    def _body(*args):
        operands = list(args)
        if partition_name is not None:
            operands.append(partition_id_tensor())
        outs = _bass_exec_p.bind(
            *operands,
            out_avals=tuple(out_avals),
            in_names=tuple(all_in_names),
            out_names=tuple(out_names),
            lowering_input_output_aliases=(),
            sim_require_finite=True,
            sim_require_nnan=True,
            nc=nc,
        )
        return tuple(outs)

    donate = tuple(range(n_params, n_params + n_outs))
    devices = jax.devices()[:n_cores]
    mesh = Mesh(_np.asarray(devices), ("core",))
    in_specs = (PartitionSpec("core"),) * (n_params + n_outs)
    out_specs = (PartitionSpec("core"),) * n_outs
    sharded = jax.jit(
        shard_map(_body, mesh=mesh, in_specs=in_specs, out_specs=out_specs,
                  check_rep=False),
        donate_argnums=donate, keep_unused=True,
    )

    def run(in_maps):
        concat_in = [
            _np.concatenate([_np.asarray(in_maps[c][n]) for c in
                             range(n_cores)], axis=0)
            for n in in_names
        ]
        concat_zeros = [
            _np.zeros((n_cores * s[0], *s[1:]), d)
            for s, d in zip(out_shapes, out_dtypes)
        ]
        out_arrs = sharded(*concat_in, *concat_zeros)
        return {
            name: _np.asarray(out_arrs[i]).reshape(n_cores, *out_shapes[i])
            for i, name in enumerate(out_names)
        }

    _CACHE[key] = run
    return run


def _host_prep(context_1, context_2, w_full, w_maxpool, w_att, w_max_att):
    """Per-core input maps (packed layouts, see _build_program_inner)."""
    maps = []
    wxf = np.ones((H, 4, PX), np.float32)
    for k, w in enumerate((w_full, w_maxpool, w_att, w_max_att)):
        wxf[:, k, 1:] = (w * w).T.astype(np.float32)
    wxf = np.ascontiguousarray(wxf)
    for b in range(B):
        c1 = context_1[b, :, :H].astype(np.float32)
        c2 = context_2[b, :, :H].astype(np.float32)
        ctp = np.ascontiguousarray(np.concatenate([c1.T, c2.T], axis=1))
        cp = np.ascontiguousarray(np.stack(
            [c1[0:128], c1[128:256], c2[0:128], c2[128:256]], axis=1))
        maps.append({"ctp": ctp, "cp": cp, "wxf": wxf})
    return maps


def _numpy_fallback(context_1, context_2, mask_1, mask_2,
                    w_full, w_maxpool, w_att, w_max_att):
    """Faithful numpy port of the reference (used only if masks aren't all-ones)."""
    NEG = -1e9
    B_, S1, H2 = context_1.shape
    h = H2 // 2
    c1 = context_1[:, :, :h].astype(np.float32)
    c2 = context_2[:, :, :h].astype(np.float32)
    m1 = mask_1.astype(bool)
    m2 = mask_2.astype(bool)

    def cosine_matrix(t1, t2):
        num = np.einsum("bih,bjh->bij", t1, t2)
        n1 = np.linalg.norm(t1, axis=-1)
        n2 = np.linalg.norm(t2, axis=-1)
        return num / (n1[:, :, None] * n2[:, None, :] + EPS)

    def masked_max(x, mask, axis, keepdims=False):
        return np.max(np.where(mask, x, NEG), axis=axis, keepdims=keepdims)

    def masked_mean(x, mask, axis, keepdims=False):
        mm = mask.astype(x.dtype)
        s = np.sum(x * mm, axis=axis, keepdims=keepdims)
        c = np.sum(np.broadcast_to(mm, x.shape), axis=axis, keepdims=keepdims)
        return s / (c + EPS)

    def masked_softmax(x, mask):
        x = np.where(mask, x, NEG)
        e = np.exp(x - x.max(-1, keepdims=True))
        return e / e.sum(-1, keepdims=True)

    def get_last(t, mask):
        idx = mask.astype(np.int32).sum(1) - 1
        return t[np.arange(t.shape[0]), idx]

    def mp_match(t1, t2, w):
        t2b = np.broadcast_to(t2, t1.shape)
        num = (t1 * t2b).sum(-1)
        den = np.linalg.norm(t1, axis=-1) * np.linalg.norm(t2b, axis=-1)
        single = (num / (den + EPS))[..., None]
        w2 = w * w
        numm = np.einsum("bsh,ph,bsh->bsp", t1, w2, t2b)
        nn1 = np.sqrt(np.einsum("bsh,ph->bsp", t1 * t1, w2))
        nn2 = np.sqrt(np.einsum("bsh,ph->bsp", t2b * t2b, w2))
        return single, numm / (nn1 * nn2 + EPS)

    def mp_match_pairwise(t1, t2, w):
        w2 = w * w
        num = np.einsum("bih,ph,bjh->bpij", t1, w2, t2)
        nn1 = np.sqrt(np.einsum("bih,ph->bpi", t1 * t1, w2))
        nn2 = np.sqrt(np.einsum("bjh,ph->bpj", t2 * t2, w2))
        res = num / (nn1[:, :, :, None] * nn2[:, :, None, :] + EPS)
        return res.transpose(0, 2, 3, 1)

    v1, v2 = [], []
    cos = cosine_matrix(c1, c2)
    v1.append(masked_max(cos, m2[:, None, :], 2, True))
    v1.append(masked_mean(cos, m2[:, None, :], 2, True))
    cosU = cos.transpose(0, 2, 1)
    v2.append(masked_max(cosU, m1[:, None, :], 2, True))
    v2.append(masked_mean(cosU, m1[:, None, :], 2, True))
    c1l = get_last(c1, m1)[:, None, :]
    c2l = get_last(c2, m2)[:, None, :]
    v1.extend(mp_match(c1, c2l, w_full))
    v2.extend(mp_match(c2, c1l, w_full))
    mm = mp_match_pairwise(c1, c2, w_maxpool)
    v1.append(masked_max(mm, m2[:, None, :, None], 2))
    v1.append(masked_mean(mm, m2[:, None, :, None], 2))
    mmT = mm.transpose(0, 2, 1, 3)
    v2.append(masked_max(mmT, m1[:, None, :, None], 2))
    v2.append(masked_mean(mmT, m1[:, None, :, None], 2))
    att2 = c2[:, None, :, :] * cos[..., None]
    att1 = c1[:, :, None, :] * cos[..., None]
    am2 = masked_softmax(att2.sum(2), m1[:, :, None])
    am1 = masked_softmax(att1.sum(1), m2[:, :, None])
    v1.extend(mp_match(c1, am2, w_att))
    v2.extend(mp_match(c2, am1, w_att))
    ax2 = masked_max(att2, m2[:, None, :, None], 2)
    ax1 = masked_max(att1, m1[:, :, None, None], 1)
    v1.extend(mp_match(c1, ax2, w_max_att))
    v2.extend(mp_match(c2, ax1, w_max_att))
    return (np.concatenate(v1, -1).astype(np.float32),
            np.concatenate(v2, -1).astype(np.float32))


def kernel(context_1, context_2, mask_1, mask_2,
           w_full, w_maxpool, w_att, w_max_att):
    context_1 = np.asarray(context_1)
    context_2 = np.asarray(context_2)
    mask_1 = np.asarray(mask_1)
    mask_2 = np.asarray(mask_2)
    w_full = np.asarray(w_full, dtype=np.float32)
    w_maxpool = np.asarray(w_maxpool, dtype=np.float32)
    w_att = np.asarray(w_att, dtype=np.float32)
    w_max_att = np.asarray(w_max_att, dtype=np.float32)

    if not (mask_1.all() and mask_2.all()):
        return _numpy_fallback(context_1, context_2, mask_1, mask_2,
                               w_full, w_maxpool, w_att, w_max_att)

    run = _get_runner(B)
    in_maps = _host_prep(context_1, context_2, w_full, w_maxpool, w_att,
                         w_max_att)
    outs = run(in_maps)
    out = outs["out"]  # (B, S, 210)
    return (np.ascontiguousarray(out[:, :, :105]),
            np.ascontiguousarray(out[:, :, 105:]))


# revision 56
# speedup vs baseline: 1.0370x; 1.0370x over previous
"""BiMPM matching kernel for Trainium2 (Bass/Tile), 8 NeuronCores.

Strategy: data-parallel over batch (B=8 -> one batch per core). Per core:
  c1, c2: (256, 128) fp32 (forward half of the contexts). All masks are
  all-ones in this problem (verified on host; numpy fallback otherwise).

v5 — restructured around three ideas:

1. ext-21 weights. Every per-perspective weight matrix gets a leading
   all-ones column ([1 | w^2] -> (h, 21)), so the unweighted "single"
   cosine and the 20-perspective "multi" match come out of ONE matmul,
   land in adjacent output columns, and share one reciprocal-norm tail.
   The same column-0 trick folds the plain cosine norms (r1c/r2c) and the
   cosine mean into the maxpool norm block / mean matmuls for free.

2. Batched rsqrt. All weighted-norm matmuls for one weight set (4x
   [128,21], both sides x both chunks) write one PSUM bank; a single
   Ln + Exp(-0.5 x) pair rsqrts the whole [128,84] bank. The ~185ns
   fixed SBUF-access cost of ACT ops is paid 14x instead of 80x.

3. lambda-ladder scans. The maxpool fused multiply+max DVE scan is the
   irreducible backbone (2.6M products/core through a 1 elem/cycle
   engine). Four perspectives {g, g+5, g+10, g+15} are packed into ONE
   [128, 4*256] scan: broadcast row k is pre-scaled by LAM^k (LAM=2^30,
   exact in the fp32 exponent), so each segment's running max dominates
   everything before it and the per-segment max is recovered exactly at
   the segment's last column (descaled by LAM^-k at harvest). 80 scans
   become 20, and the per-op PSUM-access + dispatch overhead amortizes
   4x. Scan outputs are bf16 (range 2^90 overflows f16).

Also: one perspective-scale per p (w (.) c2T serves both matmul sides),
packed input DMAs (3 loads), single [S,210] output tensor, power-mean
max-attentive path as in v4 (K=16, sign-split relu chains), engines
balanced DVE ~ scans+chains / ACT ~ transcendentals+relus+u-powers /
Pool ~ scales+X-mults+reduces.

Feature columns per side: 0 cosmax | 1 cosmean | 2 full_single |
3:23 full_multi | 23:43 mp_max | 43:63 mp_mean | 63 att_single |
64:84 att_multi | 84 amax_single | 85:105 amax_multi
"""

import numpy as np

EPS = 1e-7
S = 256  # sequence length (s1 == s2)
H = 128  # forward hidden size
P = 20   # perspectives
PX = 21  # ext: [ones | w^2]
B = 8    # batch == n_cores
C_MEAN = np.float32(1.0 / (256.0 + EPS))  # masked_mean divisor (all-ones masks)

ALPHA = 8.0   # u-side scale in the power-mean (underflow guard)
VG = 4.5      # v-side normalizer (bound on |randn| at this sample count)
KPOW = 16     # power-mean order; quotient uses S_16 and S_32
LAM = 2.0 ** 30  # maxpool scan segment ladder (exact power of two)

_CACHE = {}

_SCAN_NAME = "ANT_TTMAX_SCAN_V3"


def _register_scan():
    """Custom DVE op: out[p,k] = running max of in0[p,:k+1]*in1[p,:k+1].

    Regular mode only (the 2X table slots measurably drop odd elements on
    TRN2 silicon). out[:, -1] is the full fused multiply+max reduction; no
    accumulator companion instruction is emitted. The stock
    tensor_tensor_reduce ISA opcode has no TRN2 table row (device crash);
    the ant custom-DVE table is the supported path.
    """
    from concourse.dve_ops import DveOp, OPS, CUSTOM_DVE_SPECS, \
        _SUB_OPCODE_FOR_NAME, _CUSTOM_DVE_ROW_BASE
    from concourse.dve_spec import Spec, Src0, Src1, scan, lower, _has_src1, \
        AluOp
    from concourse.dve_uop import DveOpSpec

    if _SCAN_NAME in _SUB_OPCODE_FOR_NAME:
        return next(op for op in OPS if op.name == _SCAN_NAME)

    def _ref(in0, in1, c0, c1, c2):
        b = (np.asarray(in0, np.float32) * np.asarray(in1, np.float32))
        b = b.astype(np.float32)
        P_ = b.shape[0]
        return np.maximum.accumulate(b.reshape(P_, -1), axis=1)

    spec = Spec(body=scan(AluOp.MAX, Src0 * Src1), reference=_ref)
    row = _CUSTOM_DVE_ROW_BASE + len(OPS)
    assert row < 0x20
    shas = {}
    for ver in ("v3", "v4"):
        tmp = DveOpSpec(name=_SCAN_NAME, opcode=row,
                        uops=lower(spec, ver=ver), rd1_en=_has_src1(spec))
        shas[ver] = tmp.sha(ver)
    op = DveOp(_SCAN_NAME, spec, subdim=False, uops_sha=shas)
    OPS.append(op)
    _SUB_OPCODE_FOR_NAME[op.name] = row
    CUSTOM_DVE_SPECS[op.name] = spec
    return op


def _build_program(n_cores=8):
    import concourse.bacc as bacc
    import concourse.tile as tile
    import concourse.mybir as mybir
    import concourse.bass as bass_mod
    from concourse.masks import make_identity
    import concourse.hw_specs as hw_specs

    # Every ACT function this kernel uses (Exp, Ln, Copy, Square, Relu,
    # Identity) lives together in the "natural_log_exp_and_others" set; the
    # default per-function set chooser picks the first containing set and
    # thrashes a 1.3us table reload on every transition. Restrict the
    # choices to the combined set for this build.
    _orig_gat = hw_specs.get_activation_tables

    def _gat_combined(module_arch):
        tabs = _orig_gat(module_arch)
        keep = "natural_log_exp_and_others"
        assert keep in tabs
        return {k: (v if k == keep else set()) for k, v in tabs.items()}

    hw_specs.get_activation_tables = _gat_combined
    bacc.get_activation_tables = _gat_combined
    try:
        return _build_program_inner(n_cores, bacc, tile, mybir, bass_mod,
                                    make_identity)
    finally:
        hw_specs.get_activation_tables = _orig_gat
        bacc.get_activation_tables = _orig_gat


def _build_program_inner(n_cores, bacc, tile, mybir, bass_mod, make_identity):
    import math

    f32 = mybir.dt.float32
    f32r = mybir.dt.float32r
    bf16 = mybir.dt.bfloat16
    AL = mybir.AluOpType
    AF = mybir.ActivationFunctionType
    AX = mybir.AxisListType
    AP = bass_mod.AP

    scan_op = _register_scan()

    nc = bacc.Bacc("TRN2", target_bir_lowering=False, debug=False,
                   num_devices=n_cores)

    # ---- DRAM I/O (per core) ----
    ctp_d = nc.dram_tensor("ctp", [H, 2 * S], f32, kind="ExternalInput").ap()
    cp_d = nc.dram_tensor("cp", [128, 4, H], f32, kind="ExternalInput").ap()
    wxf_d = nc.dram_tensor("wxf", [H, 4, PX], f32, kind="ExternalInput").ap()
    out_d = nc.dram_tensor("out", [S, 210], f32, kind="ExternalOutput").ap()

    with tile.TileContext(nc) as tc:
        with tc.tile_pool(name="sb", bufs=1) as sb, \
             tc.tile_pool(name="sbr", bufs=4) as sbr, \
             tc.tile_pool(name="ps_num", bufs=2, space="PSUM") as _ps_num, \
             tc.tile_pool(name="ps_nb", bufs=2, space="PSUM") as _ps_nb, \
             tc.tile_pool(name="ps_gen", bufs=2, space="PSUM") as _ps_gen, \
             tc.tile_pool(name="dram_scratch", bufs=1, space="DRAM") as dsc:

            # PSUM tiles pad to full banks; one tag per pool so the bank
            # budget stays fixed: num 2x[128,1024] = 4 banks, nb 2x1, gen 2x1.
            class _TaggedPool:
                def __init__(self, pool, tag):
                    self.pool, self.tag = pool, tag

                def tile(self, shape, dtype):
                    return self.pool.tile(shape, dtype, tag=self.tag,
                                          name=self.tag)

            ps_num = _TaggedPool(_ps_num, "num")
            ps_nb = _TaggedPool(_ps_nb, "nb")
            ps_gen = _TaggedPool(_ps_gen, "gen")

            def scan_max(in0, in1, out):
                return nc.vector._custom_dve(scan_op, out=out, in0=in0,
                                             in1=in1)

            def flat(t, n):
                """[128, n] view of a tile's first n free elements."""
                a = t[:]
                return AP(tensor=a.tensor, offset=a.offset,
                          ap=[list(a.ap[0]), [1, n]])

            def stride_view(t, off, stride, count):
                a = t if isinstance(t, AP) else t[:]
                return AP(tensor=a.tensor, offset=a.offset + off,
                          ap=[list(a.ap[0]), [stride, count]])

            # ================= loads & constants =================
            ctp = sb.tile([H, 2 * S], f32)
            nc.sync.dma_start(ctp[:], ctp_d)
            cp = sb.tile([128, 4, H], f32)
            nc.sync.dma_start(cp[:], cp_d)
            wxf = sb.tile([H, 4, PX], f32)
            nc.sync.dma_start(wxf[:], wxf_d)
            c1T = ctp[:, 0:S]
            c2T = ctp[:, S:2 * S]

            ones_row = sb.tile([1, 128], f32)
            nc.vector.memset(ones_row[:], 1.0)
            ones_col = sb.tile([128, 1], f32)
            nc.vector.memset(ones_col[:], 1.0)
            ident = sb.tile([128, 128], f32)
            make_identity(nc, ident[:])
            QBIAS = float(math.log(VG / ALPHA))
            qbias_col = sb.tile([128, 1], f32)
            nc.vector.memset(qbias_col[:], QBIAS)
            zero_col = sb.tile([128, 1], f32)
            nc.vector.memset(zero_col[:], 0.0)
            lrowp = sb.tile([128, P], f32)
            lrow = sb.tile([128, P], f32)
            for k in range(4):
                nc.vector.memset(lrowp[:, 5 * k:5 * (k + 1)], LAM ** k)
                nc.vector.memset(lrow[:, 5 * k:5 * (k + 1)], LAM ** (-k))

            out_a = sb.tile([128, 2, 210], f32)
            nc.vector.memset(out_a[:], 0.0)
            out_all = [out_a[:, 0, :], out_a[:, 1, :]]

            # PE pstate warm-up: dependency-free chain long enough to bridge
            # the input-DMA wait so the first real (fp32) matmuls run at full
            # clock.
            for _ in range(7):
                wt = ps_gen.tile([1, 128], f32)
                nc.tensor.matmul(wt[:], ones_row[0:1, 0:1], ones_row[:],
                                 start=True, stop=True)

            # f32r-rounded copies for the cosine dot products (f32r
            # streams 1 cycle/row at free >= 256; 4x over plain fp32)
            c1Tr = sb.tile([H, S], f32r)
            c2Tr = sb.tile([H, S], f32r)
            nc.vector.tensor_copy(c1Tr[:], c1T)
            nc.vector.tensor_copy(c2Tr[:], c2T)
            pml = []
            for c in range(2):
                pm = ps_num.tile([128, 4 * S], f32)
                nc.tensor.matmul(pm[:, 0:S], c1Tr[:, c * 128:(c + 1) * 128],
                                 c2Tr[:], start=True, stop=True)
                nc.tensor.matmul(pm[:, S:2 * S], c2Tr[:, c * 128:(c + 1) * 128],
                                 c1Tr[:], start=True, stop=True)
                pml.append(pm)

            # bf16 copies (matmul operands)
            c1Tb = sb.tile([H, S], bf16)
            c2Tb = sb.tile([H, S], bf16)
            nc.gpsimd.tensor_copy(c1Tb[:], c1T)
            nc.gpsimd.tensor_copy(c2Tb[:], c2T)
            wxb = sb.tile([H, 4, PX], bf16)
            nc.gpsimd.tensor_copy(wxb[:], wxf[:])

            # squares (f32: norms feed every cosine denominator)
            c1sqT = sb.tile([H, S], f32)
            c2sqT = sb.tile([H, S], f32)
            nc.scalar.activation(c1sqT[:], c1T, AF.Square)
            nc.scalar.activation(c2sqT[:], c2T, AF.Square)

            sq_chunks = [c1sqT[:, 0:128], c1sqT[:, 128:256],
                         c2sqT[:, 0:128], c2sqT[:, 128:256]]

            # ================= batched norm blocks =================
            # R[w] = rsqrt of [c1sq_c0|c1sq_c1|c2sq_c0|c2sq_c1] x w_ext,
            # one PSUM bank + one Ln/Exp pair per weight set. Only the
            # maxpool block is on the critical path (rT -> lin -> bc ->
            # scans); the f/a/m blocks are woven into the backbone.
            def norm_block(widx, tag):
                pw = ps_nb.tile([128, 4 * PX], f32)
                for s_ in range(4):
                    nc.tensor.matmul(pw[:, s_ * PX:(s_ + 1) * PX],
                                     sq_chunks[s_], wxf[:, widx, :],
                                     start=True, stop=True)
                ln = sbr.tile([128, 4 * PX], f32, tag=f"ln{tag}", name="t",
                              bufs=2)
                nc.scalar.activation(ln[:], pw[:], AF.Ln)
                r = sb.tile([128, 4 * PX], f32, tag=f"R{tag}", name=f"R{tag}")
                nc.scalar.activation(r[:], ln[:], AF.Exp, scale=-0.5)
                return r

            R_mp = norm_block(1, "mp")   # also r1c/r2c in cols 0 of each 21

            def Rsl(r, s_, lo=0, hi=PX):
                return r[:, s_ * PX + lo:s_ * PX + hi]

            # ================= maxpool setup (critical chain) =================
            # rT[p, side*S + j] = rsqrt-weighted-norm, partition-block lambda
            # ladder baked in; one bank of transposes, one ladder-multiply,
            # one DRAM store for the broadcast loads.
            hp_mp = tc.high_priority()
            hp_mp.__enter__()
            rsc = sb.tile([128, 4 * P], f32)
            for s_ in range(4):
                nc.vector.tensor_tensor(out=rsc[:, s_ * P:(s_ + 1) * P],
                                        in0=Rsl(R_mp, s_, 1, PX),
                                        in1=lrowp[:], op=AL.mult)
            ptb = ps_gen.tile([P, 4 * 128], f32)
            for s_ in range(4):
                nc.tensor.transpose(ptb[:, s_ * 128:(s_ + 1) * 128],
                                    rsc[:, s_ * P:(s_ + 1) * P], ident[:])
            rTb = sb.tile([P, 2 * S], bf16)
            nc.scalar.copy(rTb[:], ptb[:])
            lin = dsc.tile([P, 2 * S], bf16, tag="lin", name="t")
            nc.sync.dma_start(lin[:], rTb[:])
            hp_mp.__exit__(None, None, None)

            # row norms for the cosine (1/|c1_i|, 1/|c2_j| as [1,S] rows)
            prow = ps_nb.tile([1, 2 * S], f32)
            nc.tensor.matmul(prow[0:1, 0:S], ones_col[:], c1sqT[:],
                             start=True, stop=True)
            nc.tensor.matmul(prow[0:1, S:2 * S], ones_col[:], c2sqT[:],
                             start=True, stop=True)
            lnrow = sbr.tile([1, 2 * S], f32, tag="lnrow", name="t", bufs=1)
            nc.scalar.activation(lnrow[:], prow[:], AF.Ln)
            rows_r = sb.tile([1, 2 * S], f32)
            nc.scalar.activation(rows_r[:], lnrow[:], AF.Exp, scale=-0.5)

            bc_p = ps_gen.tile([128, 2 * S], f32)
            nc.tensor.matmul(bc_p[:, 0:S], ones_row[:], rows_r[0:1, 0:S],
                             start=True, stop=True)
            nc.tensor.matmul(bc_p[:, S:2 * S], ones_row[:], rows_r[0:1, S:2 * S],
                             start=True, stop=True)
            bc_r = sb.tile([128, 2 * S], f32)
            nc.scalar.copy(bc_r[:], bc_p[:])

            cos = [sb.tile([128, S], f32, tag=f"cos{c}", name=f"cos{c}") for c in range(2)]
            cosT = [sb.tile([128, S], f32, tag=f"cosT{c}", name=f"cosT{c}") for c in range(2)]
            for c in range(2):
                nc.vector.scalar_tensor_tensor(
                    out=cos[c][:], in0=pml[c][:, 0:S],
                    scalar=Rsl(R_mp, c, 0, 1), in1=bc_r[:, S:2 * S],
                    op0=AL.mult, op1=AL.mult)
                nc.vector.scalar_tensor_tensor(
                    out=cosT[c][:], in0=pml[c][:, S:2 * S],
                    scalar=Rsl(R_mp, 2 + c, 0, 1), in1=bc_r[:, 0:S],
                    op0=AL.mult, op1=AL.mult)
                nc.vector.reduce_max(out=out_all[c][:, 0:1], in_=cos[c][:],
                                     axis=AX.X)
                nc.vector.reduce_max(out=out_all[c][:, 105:106], in_=cosT[c][:],
                                     axis=AX.X)

            # descale tiles: rD = R_slice * LAM^-(p//5), per side x chunk
            rD = []
            for s_ in range(4):
                t = sb.tile([128, P], f32, tag=f"rd{s_}", name="t")
                nc.vector.tensor_tensor(out=t[:], in0=Rsl(R_mp, s_, 1, PX),
                                        in1=lrow[:], op=AL.mult)
                rD.append(t)

            def bcast_dma(g, side):
                # side 0 loads the c2-side rT rows (lin cols S:2S)
                src = lin[:]
                t = sbr.tile([128, 4 * S], bf16, tag=f"bcd{side}", name="t",
                             bufs=2)
                nc.sync.dma_start(t[:], AP(
                    tensor=src.tensor,
                    offset=src.offset + g * 2 * S + (1 - side) * S,
                    ap=[[0, 128], [10 * S, 4], [1, S]]))
                return t

            def c2scale(p):
                t = sbr.tile([H, S], bf16, tag="c2s", name="t", bufs=20)
                nc.gpsimd.tensor_scalar_mul(t[:], c2T,
                                            wxf[:, 1, 1 + p:2 + p])
                return t

            def mp_nums(g, c2s_g):
                """16 matmuls for group g -> 4 combo tiles [128, 4*256]."""
                tiles = []
                for combo in range(4):
                    side, c = combo // 2, combo % 2
                    pn = ps_num.tile([128, 4 * S], f32)
                    for k in range(4):
                        cs = c2s_g[k]
                        if side == 0:
                            nc.tensor.matmul(pn[:, k * S:(k + 1) * S],
                                             c1Tb[:, c * 128:(c + 1) * 128],
                                             cs[:], start=True, stop=True)
                        else:
                            nc.tensor.matmul(pn[:, k * S:(k + 1) * S],
                                             cs[:, c * 128:(c + 1) * 128],
                                             c1Tb[:], start=True, stop=True)
                    tiles.append(pn)
                return tiles

            def mp_scan_harvest(g, pns, bc):
                for combo in range(4):
                    side, c = combo // 2, combo % 2
                    so = sbr.tile([128, 4 * S], bf16, tag=f"so{combo}",
                                  name="t", bufs=2)
                    scan_max(flat(pns[combo], 4 * S), flat(bc[side], 4 * S),
                             so[:])
                    base = 105 * side + 23 + g
                    nc.gpsimd.tensor_mul(
                        stride_view(out_all[c], base, 5, 4),
                        stride_view(so, S - 1, S, 4),
                        stride_view(rD[combo], g, 5, 4))

            # ================= glue generators =================
            R = {"mp": R_mp}

            def g_norm_rest():
                R["f"] = norm_block(0, "f")
                yield
                R["a"] = norm_block(2, "a")
                yield
                R["m"] = norm_block(3, "m")
                yield

            def g_chain_v():
                """v-chains: relu(+-c/VG) on DVE, ^2..^32 on ACT."""
                for side, src_off in ((0, 2), (1, 0)):  # v1 from c2, v2 from c1
                    a = sb.tile([128, 4 * H], bf16, tag=f"v{side}a", name="t")
                    b = sb.tile([128, 4 * H], bf16, tag=f"v{side}b", name="t")
                    for q in range(4):
                        c, s_ = q // 2, q % 2
                        sc = (1.0 / VG) * (1 if s_ == 0 else -1)
                        nc.vector.tensor_scalar(
                            out=a[:, q * H:(q + 1) * H],
                            in0=cp[:, src_off + c, :], scalar1=sc,
                            scalar2=0.0, op0=AL.mult, op1=AL.max)
                        yield
                    cur, nxt = a, b
                    for _ in range(5):
                        nc.scalar.activation(nxt[:], cur[:], AF.Square)
                        cur, nxt = nxt, cur
                        yield
                    # 5 squarings, ping-pong: a holds x^16, b holds x^32
                    _chains[f"v{side}16"] = a
                    _chains[f"v{side}32"] = b

            def g_chain_u(side):
                """u-chains: relu(+-ALPHA*cos^T) on DVE, powers on ACT."""
                srcs = cosT if side == 0 else cos
                a = sb.tile([128, 4 * S], bf16, tag=f"u{side}a", name="t")
                b = sb.tile([128, 4 * S], bf16, tag=f"u{side}b", name="t")
                for q in range(4):
                    c, s_ = q // 2, q % 2
                    sc = ALPHA * (1 if s_ == 0 else -1)
                    nc.vector.tensor_scalar(
                        out=a[:, q * S:(q + 1) * S], in0=srcs[c][:],
                        scalar1=sc, scalar2=0.0, op0=AL.mult, op1=AL.max)
                    yield
                cur, nxt = a, b
                for _ in range(5):
                    nc.scalar.activation(nxt[:], cur[:], AF.Square)
                    cur, nxt = nxt, cur
                    yield
                _chains[f"u{side}16"] = a
                _chains[f"u{side}32"] = b

            _chains = {}

            def g_power(side, out_t):
                """(S32/S16)^(1/16)*VG/ALPHA in transposed (h, i) layout."""
                u16, u32 = _chains[f"u{side}16"], _chains[f"u{side}32"]
                v16, v32 = _chains[f"v{side}16"], _chains[f"v{side}32"]
                lns = []
                for lvl, (uk, vk) in enumerate(((u16, v16), (u32, v32))):
                    sp = ps_nb.tile([128, S], f32)
                    for q in range(4):
                        nc.tensor.matmul(sp[:], vk[:, q * H:(q + 1) * H],
                                         uk[:, q * S:(q + 1) * S],
                                         start=(q == 0), stop=(q == 3))
                    ln = sbr.tile([128, S], f32, tag=f"pln{lvl}", name="t",
                                  bufs=2)
                    nc.scalar.activation(ln[:], sp[:], AF.Ln)
                    lns.append(ln)
                    yield
                diff = sbr.tile([128, S], f32, tag="pdif", name="t", bufs=2)
                nc.gpsimd.tensor_sub(diff[:], lns[1][:], lns[0][:])
                nc.scalar.activation(out_t[:], diff[:], AF.Exp,
                                     scale=1.0 / KPOW, bias=qbias_col[:])
                yield

            def g_softmax(side):
                """att softmax; side 0: att2 (v1 side), 1: att1."""
                cosrows = cosT if side == 0 else cos
                coff = 2 if side == 0 else 0
                tgt = att2T if side == 0 else att1T
                for c in range(2):
                    pp = ps_gen.tile([128, H], f32)
                    for jc in range(2):
                        nc.tensor.matmul(pp[:],
                                         cosrows[jc][:, c * 128:(c + 1) * 128],
                                         cp[:, coff + jc, :],
                                         start=(jc == 0), stop=(jc == 1))
                    se = sbr.tile([128, 1], f32, tag="sse", name="t")
                    ex = sbr.tile([128, H], f32, tag="sex", name="t", bufs=2)
                    nc.scalar.activation(ex[:], pp[:], AF.Exp,
                                         bias=zero_col[:], scale=1.0,
                                         accum_out=se[:])
                    yield
                    rse = sbr.tile([128, 1], f32, tag="srse", name="t")
                    nc.vector.reciprocal_approx_fast(rse[:], se[:])
                    ex2 = sbr.tile([128, H], f32, tag="sex2", name="t",
                                   bufs=2)
                    nc.scalar.mul(ex2[:], ex[:], rse[:])
                    pt = ps_gen.tile([128, 128], f32)
                    nc.tensor.transpose(pt[:], ex2[:], ident[:])
                    nc.scalar.copy(tgt[:, c * 128:(c + 1) * 128], pt[:])
                    yield

            att2T = sb.tile([H, S], f32)
            att1T = sb.tile([H, S], f32)
            amax2T = sb.tile([H, S], f32)
            amax1T = sb.tile([H, S], f32)

            def g_pos_match(side, t2T, widx, rkey, col0):
                """ext pos-match: X=(c self).(t2), singles fused via col 0."""
                cT_self = c1T if side == 0 else c2T
                xb = sbr.tile([H, S], bf16, tag="xb", name="t", bufs=2)
                nc.gpsimd.tensor_mul(xb[:], cT_self, t2T[:])
                t2sq = sbr.tile([H, S], bf16, tag="t2sq", name="t", bufs=2)
                nc.gpsimd.tensor_mul(t2sq[:], t2T[:], t2T[:])
                yield
                pn2 = ps_nb.tile([128, 2 * PX], f32)
                for c in range(2):
                    nc.tensor.matmul(pn2[:, c * PX:(c + 1) * PX],
                                     t2sq[:, c * 128:(c + 1) * 128],
                                     wxb[:, widx, :], start=True, stop=True)
                ln2 = sbr.tile([128, 2 * PX], f32, tag="pmln", name="t",
                               bufs=2)
                nc.scalar.activation(ln2[:], pn2[:], AF.Ln)
                rsqt2 = sbr.tile([128, 2 * PX], f32, tag="pmrs", name="t",
                                 bufs=2)
                nc.scalar.activation(rsqt2[:], ln2[:], AF.Exp, scale=-0.5)
                yield
                for c in range(2):
                    pnum = ps_gen.tile([128, PX], f32)
                    nc.tensor.matmul(pnum[:], xb[:, c * 128:(c + 1) * 128],
                                     wxb[:, widx, :], start=True, stop=True)
                    den = sbr.tile([128, PX], f32, tag="pmden", name="t",
                                   bufs=2)
                    nc.gpsimd.tensor_mul(den[:],
                                         rsqt2[:, c * PX:(c + 1) * PX],
                                         Rsl(R[rkey], 2 * side + c))
                    nc.vector.tensor_tensor(
                        out=out_all[c][:, 105 * side + col0:
                                       105 * side + col0 + PX],
                        in0=pnum[:], in1=den[:], op=AL.mult)
                    yield

            def g_full():
                # last-row ([1,21]) weighted norms of both sides, one bank +
                # one rsq pair (matmul operands must sit at base partition 0)
                plast = ps_nb.tile([1, 2 * PX], f32)
                nc.tensor.matmul(plast[0:1, 0:PX], c2sqT[:, S - 1:S],
                                 wxf[:, 0, :], start=True, stop=True)
                nc.tensor.matmul(plast[0:1, PX:2 * PX], c1sqT[:, S - 1:S],
                                 wxf[:, 0, :], start=True, stop=True)
                lnl = sbr.tile([1, 2 * PX], f32, tag="flnl", name="t")
                nc.scalar.activation(lnl[:], plast[:], AF.Ln)
                lastr = sbr.tile([1, 2 * PX], f32, tag="flast", name="t")
                nc.scalar.activation(lastr[:], lnl[:], AF.Exp, scale=-0.5)
                yield
                for side in range(2):
                    cTb_s = c1Tb if side == 0 else c2Tb
                    cT_o = c2T if side == 0 else c1T
                    rhs = sbr.tile([H, PX], bf16, tag="frhs", name="t", bufs=2)
                    nc.vector.tensor_scalar(out=rhs[:], in0=wxb[:, 0, :],
                                            scalar1=cT_o[:, S - 1:S],
                                            scalar2=None, op0=AL.mult)
                    bcf = ps_gen.tile([128, PX], f32)
                    nc.tensor.matmul(bcf[:], ones_row[:],
                                     lastr[0:1, side * PX:(side + 1) * PX],
                                     start=True, stop=True)
                    bcs = sbr.tile([128, PX], f32, tag="fbcs", name="t",
                                   bufs=2)
                    nc.vector.tensor_copy(bcs[:], bcf[:])
                    yield
                    for c in range(2):
                        pnum = ps_gen.tile([128, PX], f32)
                        nc.tensor.matmul(pnum[:],
                                         cTb_s[:, c * 128:(c + 1) * 128],
                                         rhs[:], start=True, stop=True)
                        den = sbr.tile([128, PX], f32, tag="fden", name="t",
                                       bufs=2)
                        nc.gpsimd.tensor_mul(den[:], Rsl(R["f"], 2 * side + c),
                                             bcs[:])
                        nc.vector.tensor_tensor(
                            out=out_all[c][:, 105 * side + 2:105 * side + 23],
                            in0=pnum[:], in1=den[:], op=AL.mult)
                        yield

            def g_mp_mean():
                for side in range(2):
                    # g2T_ext[h, q] = sum_j cother[j,h] * R_other[j, q]
                    coff = 2 if side == 0 else 0
                    cTb_s = c1Tb if side == 0 else c2Tb
                    gT = ps_nb.tile([128, PX], f32)
                    for c in range(2):
                        nc.tensor.matmul(gT[:], cp[:, coff + c, :],
                                         Rsl(R_mp, (1 - side) * 2 + c),
                                         start=(c == 0), stop=(c == 1))
                    wg = sbr.tile([H, PX], bf16, tag="wg", name="t", bufs=2)
                    nc.vector.tensor_tensor(out=wg[:], in0=wxb[:, 1, :],
                                            in1=gT[:], op=AL.mult)
                    yield
                    pmx = ps_gen.tile([128, 2 * PX], f32)
                    for c in range(2):
                        nc.tensor.matmul(pmx[:, c * PX:(c + 1) * PX],
                                         cTb_s[:, c * 128:(c + 1) * 128],
                                         wg[:], start=True, stop=True)
                    yield
                    oa = out_a[:]
                    rmp = R_mp[:]
                    pmv = pmx[:]

                    def sv(base, off, n, stride_out=210):
                        return (AP(tensor=oa.tensor,
                                   offset=oa.offset + base,
                                   ap=[list(oa.ap[0]), [stride_out, 2],
                                       [1, n]]),
                                AP(tensor=pmv.tensor,
                                   offset=pmv.offset + off,
                                   ap=[list(pmv.ap[0]), [PX, 2], [1, n]]),
                                AP(tensor=rmp.tensor,
                                   offset=rmp.offset + 2 * side * PX + off,
                                   ap=[list(rmp.ap[0]), [PX, 2], [1, n]]))

                    d_, p_, r_ = sv(105 * side + 43, 1, 20)
                    nc.vector.scalar_tensor_tensor(
                        out=d_, in0=p_, scalar=float(C_MEAN), in1=r_,
                        op0=AL.mult, op1=AL.mult)
                    d_, p_, r_ = sv(105 * side + 1, 0, 1)
                    nc.vector.scalar_tensor_tensor(
                        out=d_, in0=p_, scalar=float(C_MEAN), in1=r_,
                        op0=AL.mult, op1=AL.mult)
                    yield

            def out_dst(lo, n):
                a = out_d
                return AP(tensor=a.tensor, offset=a.offset + lo,
                          ap=[[210, 128], [128 * 210, 2], [1, n]])

            def g_store_early(side):
                lo = 105 * side + 43
                nc.sync.dma_start(out_dst(lo, 62), out_a[:, :, lo:lo + 62])
                yield

            # ================= weave =================
            from collections import deque
            pending = deque()

            def pump(n):
                for _ in range(n):
                    emitted = False
                    while pending and not emitted:
                        g_ = pending.popleft()
                        try:
                            next(g_)
                            pending.insert(min(2, len(pending)), g_)
                            emitted = True
                        except StopIteration:
                            pass
                    if not pending and not emitted:
                        return

            def run_all(gen):
                for _ in gen:
                    pass

            # schedule: per-group event lists (run between scan groups)
            events = {
                0: [g_norm_rest(), g_chain_v(), g_chain_u(0), g_softmax(0)],
                1: [g_chain_u(1), g_full(), g_pos_match(0, att2T, 2, "a", 63)],
                2: [g_power(0, amax2T), g_softmax(1), g_mp_mean()],
                3: [g_pos_match(0, amax2T, 3, "m", 84), g_power(1, amax1T),
                    g_pos_match(1, att1T, 2, "a", 63)],
                4: [g_pos_match(1, amax1T, 3, "m", 84)],
            }

            # ---- maxpool backbone ----
            bc = (bcast_dma(0, 0), bcast_dma(0, 1))
            c2s_g = [c2scale(0 + 5 * k) for k in range(4)]
            pns = mp_nums(0, c2s_g)
            for g in range(5):
                for ev in events.get(g, ()):
                    if callable(ev):
                        ev()
                    else:
                        pending.append(ev)
                if g + 1 < 5:
                    nbc = (bcast_dma(g + 1, 0), bcast_dma(g + 1, 1))
                    c2s_n = [c2scale(g + 1 + 5 * k) for k in range(4)]
                pump(8)
                mp_scan_harvest(g, pns, bc)
                if g + 1 < 5:
                    pns = mp_nums(g + 1, c2s_n)
                    bc = nbc
                pump(10)
                if g == 3:
                    # v1's 43:105 block is complete once the g<=3 glue
                    # (mean, att-m1, amax-m1) has drained
                    pump(100000)
                    run_all(g_store_early(0))
                if g == 4:
                    pump(100000)
                    run_all(g_store_early(1))
            pump(1000)

            # ================= final stores =================
            nc.sync.dma_start(out_dst(0, 43), out_a[:, :, 0:43])
            nc.sync.dma_start(out_dst(105, 43), out_a[:, :, 105:148])

    nc.finalize()
    return nc


def _get_program(n_cores=8):
    key = ("prog", n_cores)
    if key not in _CACHE:
        _CACHE[key] = _build_program(n_cores)
    return _CACHE[key]


def _get_runner(n_cores=8):
    """Build (once) a cached jitted executor: fn(in_maps) -> per-core outputs.

    Mirrors concourse.bass2jax.run_bass_via_pjrt's multi-core path, but keeps
    the jitted shard_map so repeat calls skip tracing/compile-cache lookups.
    """
    key = ("runner", n_cores)
    if key in _CACHE:
        return _CACHE[key]

    import jax
    import numpy as _np
    from jax.experimental.shard_map import shard_map
    from jax.sharding import Mesh, PartitionSpec
    import concourse.mybir as mybir
    from concourse.bass2jax import (_bass_exec_p, install_neuronx_cc_hook,
                                    partition_id_tensor)

    nc = _get_program(n_cores)
    install_neuronx_cc_hook()
    partition_name = (nc.partition_id_tensor.name
                      if nc.partition_id_tensor else None)

    in_names, out_names, out_shapes, out_dtypes = [], [], [], []
    for alloc in nc.m.functions[0].allocations:
        if not isinstance(alloc, mybir.MemoryLocationSet):
            continue
        name = alloc.memorylocations[0].name
        if alloc.kind == "ExternalInput":
            if name != partition_name:
                in_names.append(name)
        elif alloc.kind == "ExternalOutput":
            out_names.append(name)
            out_shapes.append(tuple(alloc.tensor_shape))
            out_dtypes.append(mybir.dt.np(alloc.dtype))
    n_params = len(in_names)
    n_outs = len(out_names)
    out_avals = [jax.core.ShapedArray(s, d)
                 for s, d in zip(out_shapes, out_dtypes)]
    all_in_names = list(in_names) + list(out_names)
    if partition_name is not None:
        all_in_names.append(partition_name)

    def _body(*args):
        operands = list(args)
        if partition_name is not None:
            operands.append(partition_id_tensor())
        outs = _bass_exec_p.bind(
            *operands,
            out_avals=tuple(out_avals),
            in_names=tuple(all_in_names),
            out_names=tuple(out_names),
            lowering_input_output_aliases=(),
            sim_require_finite=True,
            sim_require_nnan=True,
            nc=nc,
        )
        return tuple(outs)

    donate = tuple(range(n_params, n_params + n_outs))
    devices = jax.devices()[:n_cores]
    mesh = Mesh(_np.asarray(devices), ("core",))
    in_specs = (PartitionSpec("core"),) * (n_params + n_outs)
    out_specs = (PartitionSpec("core"),) * n_outs
    sharded = jax.jit(
        shard_map(_body, mesh=mesh, in_specs=in_specs, out_specs=out_specs,
                  check_rep=False),
        donate_argnums=donate, keep_unused=True,
    )

    def run(in_maps):
        concat_in = [
            _np.concatenate([_np.asarray(in_maps[c][n]) for c in
                             range(n_cores)], axis=0)
            for n in in_names
        ]
        concat_zeros = [
            _np.zeros((n_cores * s[0], *s[1:]), d)
            for s, d in zip(out_shapes, out_dtypes)
        ]
        out_arrs = sharded(*concat_in, *concat_zeros)
        return {
            name: _np.asarray(out_arrs[i]).reshape(n_cores, *out_shapes[i])
            for i, name in enumerate(out_names)
        }

    _CACHE[key] = run
    return run


def _host_prep(context_1, context_2, w_full, w_maxpool, w_att, w_max_att):
    """Per-core input maps (packed layouts, see _build_program_inner)."""
    maps = []
    wxf = np.ones((H, 4, PX), np.float32)
    for k, w in enumerate((w_full, w_maxpool, w_att, w_max_att)):
        wxf[:, k, 1:] = (w * w).T.astype(np.float32)
    wxf = np.ascontiguousarray(wxf)
    for b in range(B):
        c1 = context_1[b, :, :H].astype(np.float32)
        c2 = context_2[b, :, :H].astype(np.float32)
        ctp = np.ascontiguousarray(np.concatenate([c1.T, c2.T], axis=1))
        cp = np.ascontiguousarray(np.stack(
            [c1[0:128], c1[128:256], c2[0:128], c2[128:256]], axis=1))
        maps.append({"ctp": ctp, "cp": cp, "wxf": wxf})
    return maps


def _numpy_fallback(context_1, context_2, mask_1, mask_2,
                    w_full, w_maxpool, w_att, w_max_att):
    """Faithful numpy port of the reference (used only if masks aren't all-ones)."""
    NEG = -1e9
    B_, S1, H2 = context_1.shape
    h = H2 // 2
    c1 = context_1[:, :, :h].astype(np.float32)
    c2 = context_2[:, :, :h].astype(np.float32)
    m1 = mask_1.astype(bool)
    m2 = mask_2.astype(bool)

    def cosine_matrix(t1, t2):
        num = np.einsum("bih,bjh->bij", t1, t2)
        n1 = np.linalg.norm(t1, axis=-1)
        n2 = np.linalg.norm(t2, axis=-1)
        return num / (n1[:, :, None] * n2[:, None, :] + EPS)

    def masked_max(x, mask, axis, keepdims=False):
        return np.max(np.where(mask, x, NEG), axis=axis, keepdims=keepdims)

    def masked_mean(x, mask, axis, keepdims=False):
        mm = mask.astype(x.dtype)
        s = np.sum(x * mm, axis=axis, keepdims=keepdims)
        c = np.sum(np.broadcast_to(mm, x.shape), axis=axis, keepdims=keepdims)
        return s / (c + EPS)

    def masked_softmax(x, mask):
        x = np.where(mask, x, NEG)
        e = np.exp(x - x.max(-1, keepdims=True))
        return e / e.sum(-1, keepdims=True)

    def get_last(t, mask):
        idx = mask.astype(np.int32).sum(1) - 1
        return t[np.arange(t.shape[0]), idx]

    def mp_match(t1, t2, w):
        t2b = np.broadcast_to(t2, t1.shape)
        num = (t1 * t2b).sum(-1)
        den = np.linalg.norm(t1, axis=-1) * np.linalg.norm(t2b, axis=-1)
        single = (num / (den + EPS))[..., None]
        w2 = w * w
        numm = np.einsum("bsh,ph,bsh->bsp", t1, w2, t2b)
        nn1 = np.sqrt(np.einsum("bsh,ph->bsp", t1 * t1, w2))
        nn2 = np.sqrt(np.einsum("bsh,ph->bsp", t2b * t2b, w2))
        return single, numm / (nn1 * nn2 + EPS)

    def mp_match_pairwise(t1, t2, w):
        w2 = w * w
        num = np.einsum("bih,ph,bjh->bpij", t1, w2, t2)
        nn1 = np.sqrt(np.einsum("bih,ph->bpi", t1 * t1, w2))
        nn2 = np.sqrt(np.einsum("bjh,ph->bpj", t2 * t2, w2))
        res = num / (nn1[:, :, :, None] * nn2[:, :, None, :] + EPS)
        return res.transpose(0, 2, 3, 1)

    v1, v2 = [], []
    cos = cosine_matrix(c1, c2)
    v1.append(masked_max(cos, m2[:, None, :], 2, True))
    v1.append(masked_mean(cos, m2[:, None, :], 2, True))
    cosU = cos.transpose(0, 2, 1)
    v2.append(masked_max(cosU, m1[:, None, :], 2, True))
    v2.append(masked_mean(cosU, m1[:, None, :], 2, True))
    c1l = get_last(c1, m1)[:, None, :]
    c2l = get_last(c2, m2)[:, None, :]
    v1.extend(mp_match(c1, c2l, w_full))
    v2.extend(mp_match(c2, c1l, w_full))
    mm = mp_match_pairwise(c1, c2, w_maxpool)
    v1.append(masked_max(mm, m2[:, None, :, None], 2))
    v1.append(masked_mean(mm, m2[:, None, :, None], 2))
    mmT = mm.transpose(0, 2, 1, 3)
    v2.append(masked_max(mmT, m1[:, None, :, None], 2))
    v2.append(masked_mean(mmT, m1[:, None, :, None], 2))
    att2 = c2[:, None, :, :] * cos[..., None]
    att1 = c1[:, :, None, :] * cos[..., None]
    am2 = masked_softmax(att2.sum(2), m1[:, :, None])
    am1 = masked_softmax(att1.sum(1), m2[:, :, None])
    v1.extend(mp_match(c1, am2, w_att))
    v2.extend(mp_match(c2, am1, w_att))
    ax2 = masked_max(att2, m2[:, None, :, None], 2)
    ax1 = masked_max(att1, m1[:, :, None, None], 1)
    v1.extend(mp_match(c1, ax2, w_max_att))
    v2.extend(mp_match(c2, ax1, w_max_att))
    return (np.concatenate(v1, -1).astype(np.float32),
            np.concatenate(v2, -1).astype(np.float32))


def kernel(context_1, context_2, mask_1, mask_2,
           w_full, w_maxpool, w_att, w_max_att):
    context_1 = np.asarray(context_1)
    context_2 = np.asarray(context_2)
    mask_1 = np.asarray(mask_1)
    mask_2 = np.asarray(mask_2)
    w_full = np.asarray(w_full, dtype=np.float32)
    w_maxpool = np.asarray(w_maxpool, dtype=np.float32)
    w_att = np.asarray(w_att, dtype=np.float32)
    w_max_att = np.asarray(w_max_att, dtype=np.float32)

    if not (mask_1.all() and mask_2.all()):
        return _numpy_fallback(context_1, context_2, mask_1, mask_2,
                               w_full, w_maxpool, w_att, w_max_att)

    run = _get_runner(B)
    in_maps = _host_prep(context_1, context_2, w_full, w_maxpool, w_att,
                         w_max_att)
    outs = run(in_maps)
    out = outs["out"]  # (B, S, 210)
    return (np.ascontiguousarray(out[:, :, :105]),
            np.ascontiguousarray(out[:, :, 105:]))
